# revision 4
# baseline (speedup 1.0000x reference)
"""NetGINE (4-layer GIN message passing) on 8 Trainium2 NeuronCores.

Sharding: nodes/edges sharded by destination across 8 cores (6400 padded node
slots per core). Per layer:
  - bond encoder e = relu(ea @ be1) @ be2 on TensorE (2-group stacked chain)
  - h[src] gathered per edge from a replicated padded node table in HBM via
    gpsimd dma_gather (4 SWDGE queues, table split in halves for int16 idx)
  - msg = relu(h_src + e); segment-sum via one-hot matmuls accumulating in PSUM
  - node MLP + BN on transposed [64, nodes] tiles
  - AllGather of the [6400, 64] shard to rebuild the replicated table
Pooling: per-tile one-hot matmuls into 4 graph windows + AllReduce; head MLP
computed redundantly on every core.
"""

import os
import numpy as np
import ml_dtypes

BF16 = np.dtype(ml_dtypes.bfloat16)

N, E, G, DIM, XF, EF = 50000, 800000, 512, 64, 28, 3
NCORES = 8
NLOC = 6400              # padded node slots per core
NPAD = NCORES * NLOC     # 51200
TILES = NLOC // 128      # 50
HALF = NPAD // 2         # 25600 (table split for int16 gather indices)
TPC = 4                  # tiles per chunk
NLAYERS = 4
GWIN = 4                 # 128-graph pooling windows

_DEBUG = os.environ.get("BASSGIN_DEBUG", "0") == "1"


# ---------------------------------------------------------------- host prep --

def _plan_nodes(batch):
    """Assign nodes to per-core padded slots; no 128-slot tile may span a
    128-graph window boundary."""
    slot2node = np.full((NCORES, NLOC), -1, np.int64)
    node2pad = np.full(N, -1, np.int64)
    per_core = N // NCORES  # 6250
    for c in range(NCORES):
        nodes = np.arange(c * per_core, (c + 1) * per_core)
        wins = batch[nodes] // 128
        change = np.nonzero(np.diff(wins))[0] + 1
        bounds = [0] + list(change) + [len(nodes)]
        s = 0
        for i in range(len(bounds) - 1):
            lo, hi = bounds[i], bounds[i + 1]
            if i > 0 and s % 128 != 0:
                s += 128 - (s % 128)
            cnt = hi - lo
            assert s + cnt <= NLOC, "node padding overflow"
            slot2node[c, s:s + cnt] = nodes[lo:hi]
            node2pad[nodes[lo:hi]] = c * NLOC + s + np.arange(cnt)
            s += cnt
    return slot2node, node2pad


def _prep(inputs):
    x = np.asarray(inputs["x"], np.float32)
    edge_attr = np.asarray(inputs["edge_attr"], np.float32)
    edge_index = np.asarray(inputs["edge_index"], np.int64)
    batch = np.asarray(inputs["batch"], np.int64)

    slot2node, node2pad = _plan_nodes(batch)

    src_p = node2pad[edge_index[0]]
    dst_p = node2pad[edge_index[1]]
    core = dst_p // NLOC
    dslot = dst_p % NLOC
    tile_of = dslot // 128
    drel = dslot % 128
    grp_b = (src_p >= HALF).astype(np.int64)   # 0 = A, 1 = B

    key = (core * TILES + tile_of) * 2 + grp_b
    counts = np.bincount(key, minlength=NCORES * TILES * 2).reshape(NCORES, TILES, 2)
    BA = max(int(np.ceil(counts[:, :, 0].max() / 128)), 1)
    BB = max(int(np.ceil(counts[:, :, 1].max() / 128)), 1)
    if (BA + BB) % 2 == 1:
        BB += 1
    NBT = BA + BB
    NBLK = TILES * NBT
    SLOTS = NBLK * 128
    assert BA <= 15 and BB <= 15, (BA, BB)

    chunk_tiles = [list(range(t, min(t + TPC, TILES)))
                   for t in range(0, TILES, TPC)]

    slot_base = {}
    s0 = 0
    for tl in chunk_tiles:
        for ti in tl:
            slot_base[(ti, 0)] = s0
            s0 += BA * 128
        for ti in tl:
            slot_base[(ti, 1)] = s0
            s0 += BB * 128
    assert s0 == SLOTS

    # slot of each edge: base of its (tile, group) + rank within that list
    order = np.argsort(key, kind="stable")
    ends = np.cumsum(counts.reshape(-1))
    starts = ends - counts.reshape(-1)
    rank = np.empty(E, np.int64)
    rank[order] = np.arange(E) - starts[key[order]]
    base_arr = np.zeros((NCORES, TILES, 2), np.int64)
    for ti in range(TILES):
        for g in range(2):
            base_arr[:, ti, g] = slot_base[(ti, g)]
    slot_of_edge = base_arr.reshape(-1)[key] + rank

    idx_val = np.where(grp_b == 0, src_p, src_p - HALF)

    gidx = np.zeros((NCORES, 128, SLOTS // 16), np.int16)
    dstrel = np.full((NCORES, 128, SLOTS // 128), -1.0, np.float32)
    NGRP = (SLOTS // 128) // 4
    easl = np.zeros((NCORES, SLOTS, EF), np.float32)
    for c in range(NCORES):
        m = core == c
        sl = slot_of_edge[m]
        dstrel[c][sl % 128, sl // 128] = drel[m]
        easl[c][sl] = edge_attr[m]
        iv = np.zeros(SLOTS, np.int64)
        iv[sl] = idx_val[m]
        col0 = 0
        for tl in chunk_tiles:
            for g, BG in ((0, BA), (1, BB)):
                L = len(tl) * BG * 128
                s_lo = slot_base[(tl[0], g)]
                j = np.arange(L)
                gidx[c][j % 16, col0 + j // 16] = iv[s_lo:s_lo + L].astype(np.int16)
                col0 += L // 16
        gidx[c] = np.tile(gidx[c][:16], (8, 1))

    dstrelT = np.zeros((NCORES, 8, NGRP, 128), np.float32)
    for c in range(NCORES):
        dq = dstrel[c].T.reshape(NGRP, 4, 128)   # [g, q, p]
        dstrelT[c, 0:4] = dq.transpose(1, 0, 2)
        dstrelT[c, 4] = 1.0
    tgen_rhs = np.zeros((8, 512), np.float32)
    for q in range(4):
        tgen_rhs[q, 128 * q:128 * (q + 1)] = 1.0
    tgen_rhs[4] = -np.tile(np.arange(128, dtype=np.float32), 4)

    # eaT2 stacked pairing: unit u covers slots [256u,256u+128) top, +128 bottom
    easl_u = easl.reshape(NCORES, SLOTS // 256, 2, 128, EF)
    eaT2 = np.zeros((NCORES, 2 * EF, SLOTS // 2), BF16)
    for c in range(NCORES):
        eaT2[c, :EF] = easl_u[c, :, 0].transpose(2, 0, 1).reshape(EF, -1).astype(BF16)
        eaT2[c, EF:] = easl_u[c, :, 1].transpose(2, 0, 1).reshape(EF, -1).astype(BF16)

    # node-side tensors
    xpad = np.zeros((N, DIM), np.float32)
    xpad[:, :XF] = x
    T1 = np.zeros((NPAD, DIM), np.float32)
    flat = slot2node.reshape(-1)
    valid = flat >= 0
    T1[valid] = xpad[flat[valid]]
    hT0 = np.zeros((NCORES, DIM, NLOC), np.float32)
    gid4 = np.full((NCORES, 128, GWIN * TILES), -1e9, np.float32)
    realmask = np.zeros((NCORES, 128, TILES), np.float32)
    for c in range(NCORES):
        sn = slot2node[c]
        v = sn >= 0
        hT0[c][:, v.nonzero()[0]] = xpad[sn[v]].T
        gid = np.full(NLOC, -1e9)
        gid[v] = batch[sn[v]]
        for w in range(GWIN):
            gid4[c][:, w * TILES:(w + 1) * TILES] = \
                (gid - 128 * w).reshape(TILES, 128).T
        realmask[c] = v.astype(np.float32).reshape(TILES, 128).T

    def padw(a, r, cc):
        out = np.zeros((r, cc), np.float32)
        a = np.asarray(a, np.float32)
        out[:a.shape[0], :a.shape[1]] = a
        return out

    wb = {}
    for li, p in ((1, "c1"), (2, "c2"), (3, "c3")):
        be1 = padw(inputs[f"{p}_be1"], EF, DIM)
        be2 = padw(inputs[f"{p}_be2"], DIM, DIM)
        be1_2 = np.zeros((2 * EF, 128), np.float32)
        be1_2[:EF, :DIM] = be1
        be1_2[EF:, DIM:] = be1
        be2_2 = np.zeros((128, 128), np.float32)
        be2_2[:DIM, :DIM] = be2
        be2_2[DIM:, DIM:] = be2
        wb[f"be1_{li}"] = be1_2.astype(BF16)
        wb[f"be2_{li}"] = be2_2.astype(BF16)
        wb[f"m1_{li}"] = padw(inputs[f"{p}_m1"], DIM, DIM)
        wb[f"m2_{li}"] = padw(inputs[f"{p}_m2"], DIM, DIM)
    eps = [float(np.asarray(inputs[f"{p}_eps"]).reshape(-1)[0])
           for p in ("c1", "c2", "c3")]
    epsv = np.array([[eps[0]], [eps[1]], [eps[2]], [eps[2]]], np.float32)

    common = {
        "t0": T1,
        "iota": np.tile(np.arange(128, dtype=np.float32), (128, 1)),
        "iotab": np.tile(np.arange(128, dtype=np.float32), (128, 1)).astype(BF16),
        "idbf": np.eye(128, dtype=np.float32).astype(BF16),
        "idf32": np.eye(128, dtype=np.float32),
        "epsv": epsv,
        "tgen_rhs": tgen_rhs.astype(BF16),
        "fc1_w": np.asarray(inputs["fc1_w"], np.float32).reshape(2, 128, DIM)
                   .transpose(1, 0, 2).copy(),
        "fc1_b": np.asarray(inputs["fc1_b"], np.float32).reshape(DIM, 1),
        "fc2_w": np.asarray(inputs["fc2_w"], np.float32),
        "fc2_b": np.asarray(inputs["fc2_b"], np.float32).reshape(DIM, 1),
        "fc3_w": np.asarray(inputs["fc3_w"], np.float32),
        "fc3_b": np.asarray(inputs["fc3_b"], np.float32).reshape(DIM, 1),
        "fc4_w": np.asarray(inputs["fc4_w"], np.float32),
        "fc4_b": np.asarray(inputs["fc4_b"], np.float32).reshape(1, 1),
    }
    common.update(wb)
    for i in range(1, 5):
        for s in "gbmv":
            common[f"bn{i}_{s}"] = np.asarray(inputs[f"bn{i}_{s}"],
                                              np.float32).reshape(DIM, 1)

    in_maps = []
    for c in range(NCORES):
        m = dict(common)
        m["gidx"] = gidx[c]
        m["dstrelT"] = dstrelT[c].astype(BF16)
        m["eaT2"] = eaT2[c]
        m["hT0"] = hT0[c]
        m["gid4"] = gid4[c]
        m["realmask"] = realmask[c]
        in_maps.append(m)

    struct = dict(BA=BA, BB=BB, NBT=NBT, NBLK=NBLK, SLOTS=SLOTS,
                  chunk_tiles=chunk_tiles, slot_base=slot_base)
    return in_maps, struct


# ------------------------------------------------------------- bass program --

def _build(struct):
    from concourse import bacc, tile, mybir
    f32, bf16, i16 = mybir.dt.float32, mybir.dt.bfloat16, mybir.dt.int16
    Alu = mybir.AluOpType
    Act = mybir.ActivationFunctionType

    BA, BB, NBT = struct["BA"], struct["BB"], struct["NBT"]
    SLOTS = struct["SLOTS"]
    chunk_tiles = struct["chunk_tiles"]
    NB = TPC * NBT            # max blocks per chunk

    nc = bacc.Bacc("TRN2", target_bir_lowering=False, debug=False,
                   num_devices=NCORES, num_swdge_queues=4)

    def din(name, shape, dt=f32):
        return nc.dram_tensor(name, shape, dt, kind="ExternalInput")

    t0 = din("t0", [NPAD, DIM])
    gidx_d = din("gidx", [128, SLOTS // 16], i16)
    NGRP = (SLOTS // 128) // 4
    dstrelT_d = din("dstrelT", [8, NGRP, 128], bf16)
    tgen_d = din("tgen_rhs", [8, 512], bf16)
    eaT2_d = din("eaT2", [2 * EF, SLOTS // 2], bf16)
    hT0_d = din("hT0", [DIM, NLOC])
    gid4_d = din("gid4", [128, GWIN * TILES])
    realmask_d = din("realmask", [128, TILES])
    iota_d = din("iota", [128, 128])
    iotab_d = din("iotab", [128, 128], bf16)
    idbf_d = din("idbf", [128, 128], bf16)
    idf32_d = din("idf32", [128, 128])
    epsv_d = din("epsv", [4, 1])
    wdict = {}
    for li in (1, 2, 3):
        wdict[f"be1_{li}"] = din(f"be1_{li}", [2 * EF, 128], bf16)
        wdict[f"be2_{li}"] = din(f"be2_{li}", [128, 128], bf16)
        wdict[f"m1_{li}"] = din(f"m1_{li}", [DIM, DIM])
        wdict[f"m2_{li}"] = din(f"m2_{li}", [DIM, DIM])
    for i in range(1, 5):
        for s in "gbmv":
            wdict[f"bn{i}_{s}"] = din(f"bn{i}_{s}", [DIM, 1])
    fc1_w = din("fc1_w", [128, 2, DIM])
    fc2_w = din("fc2_w", [DIM, DIM])
    fc3_w = din("fc3_w", [DIM, DIM])
    fc4_w = din("fc4_w", [DIM, 1])
    fcb_d = {"b1": din("fc1_b", [DIM, 1]), "b2": din("fc2_b", [DIM, 1]),
             "b3": din("fc3_b", [DIM, 1]), "b4": din("fc4_b", [1, 1])}

    out_d = nc.dram_tensor("out", [1, G], f32, kind="ExternalOutput")
    bounce = [nc.dram_tensor(f"bounce{l}", [NLOC, DIM], f32)
              for l in range(NLAYERS)]
    dbg = [nc.dram_tensor(f"dbg{l}", [NLOC, DIM], f32, kind="ExternalOutput")
           for l in range(NLAYERS)] if _DEBUG else None
    dbg_agg = nc.dram_tensor("dbg_agg", [DIM, NLOC], f32,
                             kind="ExternalOutput") if _DEBUG else None
    dbg_z = nc.dram_tensor("dbg_z", [DIM, NLOC], f32,
                           kind="ExternalOutput") if _DEBUG else None
    dbg_par = nc.dram_tensor("dbg_par", [DIM, 12], f32,
                             kind="ExternalOutput") if _DEBUG else None
    tables = [t0] + [nc.dram_tensor(f"T{l}", [NPAD, DIM], f32, addr_space="Shared")
                     for l in (1, 2, 3)]
    arin_d = nc.dram_tensor("arin", [128, GWIN, 260], f32)
    arout_d = nc.dram_tensor("arout", [128, GWIN, 260], f32, addr_space="Shared")

    qctr = [0]

    def next_q():
        q = qctr[0] % 4
        qctr[0] += 1
        return q

    with tile.TileContext(nc) as tc:
        with tc.tile_pool(name="res", bufs=1) as res, \
             tc.tile_pool(name="hsrcp", bufs=3) as hsrcp, \
             tc.tile_pool(name="msgp", bufs=2) as msgp, \
             tc.tile_pool(name="e1p", bufs=2) as e1p, \
             tc.tile_pool(name="eap", bufs=1) as eap, \
             tc.tile_pool(name="gixp", bufs=2) as gixp, \
             tc.tile_pool(name="smallp", bufs=2) as smallp, \
             tc.tile_pool(name="psA", bufs=3, space="PSUM") as psA, \
             tc.tile_pool(name="psB", bufs=1, space="PSUM") as psB, \
             tc.tile_pool(name="psP", bufs=2, space="PSUM") as psP, \
             tc.tile_pool(name="psG", bufs=2, space="PSUM") as psG:

            # ---------------- residents
            def load(name, shape, dt, dram):
                tl_ = res.tile(shape, dt, tag=name)
                nc.sync.dma_start(out=tl_[:], in_=dram[:])
                return tl_

            tgen_sb = load("tgen", [8, 512], bf16, tgen_d)
            iota_sb = load("iota", [128, 128], f32, iota_d)
            iotab_sb = load("iotab", [128, 128], bf16, iotab_d)
            idbf_sb = load("idbf", [128, 128], bf16, idbf_d)
            idf_sb = load("idf", [128, 128], f32, idf32_d)
            gid4_sb = load("gid4", [128, GWIN * TILES], f32, gid4_d)
            rm_sb = load("rm", [128, TILES], f32, realmask_d)
            w_sb = {k: load(f"w_{k}", list(d.shape), d.dtype, d)
                    for k, d in wdict.items()}
            fc1w_sb = load("fc1w", [128, 2, DIM], f32, fc1_w)
            fc2w_sb = load("fc2w", [DIM, DIM], f32, fc2_w)
            fc3w_sb = load("fc3w", [DIM, DIM], f32, fc3_w)
            fc4w_sb = load("fc4w", [DIM, 1], f32, fc4_w)
            fcb_sb = {k: load(f"fcb{k}", list(d.shape), f32, d)
                      for k, d in fcb_d.items()}
            hT = load("hT", [DIM, NLOC], f32, hT0_d)
            aggT = res.tile([DIM, NLOC], bf16, tag="aggT")

            # eps broadcast [64,1] per layer: (1+eps)
            eps1p = []
            for l in range(NLAYERS):
                e0 = res.tile([1, 1], f32, tag=f"eps0_{l}")
                nc.sync.dma_start(out=e0[:], in_=epsv_d[l:l + 1, :])
                eb = res.tile([DIM, 1], f32, tag=f"epsb{l}")
                nc.gpsimd.partition_broadcast(eb[:], e0[:], channels=DIM)
                e1 = res.tile([DIM, 1], f32, tag=f"eps1p{l}")
                nc.vector.tensor_scalar_add(e1[:], eb[:], 1.0)
                eps1p.append(e1)

            # bn params -> scale g', shift b'
            bn_s, bn_t = [], []
            for i in range(1, 5):
                v = w_sb[f"bn{i}_v"]; gg = w_sb[f"bn{i}_g"]
                bb = w_sb[f"bn{i}_b"]; mm = w_sb[f"bn{i}_m"]
                ve = res.tile([DIM, 1], f32, tag=f"bnve{i}")
                nc.vector.tensor_scalar_add(ve[:], v[:], 1e-5)
                sq = res.tile([DIM, 1], f32, tag=f"bnsq{i}")
                nc.scalar.activation(sq[:], ve[:], Act.Sqrt)
                inv = res.tile([DIM, 1], f32, tag=f"bninv{i}")
                nc.vector.reciprocal(inv[:], sq[:])
                gp = res.tile([DIM, 1], f32, tag=f"bngp{i}")
                nc.vector.tensor_mul(gp[:], gg[:], inv[:])
                tt = res.tile([DIM, 1], f32, tag=f"bntt{i}")
                nc.vector.tensor_mul(tt[:], mm[:], gp[:])
                bp = res.tile([DIM, 1], f32, tag=f"bnbp{i}")
                nc.vector.tensor_sub(bp[:], bb[:], tt[:])
                bn_s.append(gp); bn_t.append(bp)

            # ---------------- layers
            for l in range(NLAYERS):
                wl = min(l + 1, 3)
                be1 = w_sb[f"be1_{wl}"]; be2 = w_sb[f"be2_{wl}"]
                m1 = w_sb[f"m1_{wl}"]; m2 = w_sb[f"m2_{wl}"]
                tbl = tables[l]

                for ci, tl in enumerate(chunk_tiles):
                    ntl = len(tl)
                    nb = ntl * NBT
                    ncols = nb * 64
                    s0 = struct["slot_base"][(tl[0], 0)]
                    assert s0 % 256 == 0
                    la = ntl * BA * 128
                    lb = ntl * BB * 128
                    # gather idx staging + gathers (A then B table half)
                    gix = gixp.tile([128, TPC * NBT * 8], i16, tag="gix")
                    nc.sync.dma_start(out=gix[:, 0:nb * 8],
                                      in_=gidx_d[:, s0 // 16:(s0 + la + lb) // 16])
                    hsrc = hsrcp.tile([128, NB, DIM], f32, tag="hsrc")
                    nc.gpsimd.dma_gather(
                        out_ap=hsrc[:, 0:ntl * BA, :], in_ap=tbl[0:HALF, :],
                        idxs_ap=gix[:, 0:la // 16],
                        num_idxs=la, num_idxs_reg=la, elem_size=DIM,
                        single_packet=False, queue_num=next_q())
                    nc.gpsimd.dma_gather(
                        out_ap=hsrc[:, ntl * BA:nb, :], in_ap=tbl[HALF:NPAD, :],
                        idxs_ap=gix[:, la // 16:(la + lb) // 16],
                        num_idxs=lb, num_idxs_reg=lb, elem_size=DIM,
                        single_packet=False, queue_num=next_q())

                    # bond encoder stage 1 for this chunk
                    c0 = s0 // 2
                    ea_sb = eap.tile([2 * EF, TPC * NBT * 64], bf16, tag="ea")
                    nc.sync.dma_start(out=ea_sb[:, 0:ncols],
                                      in_=eaT2_d[:, c0:c0 + ncols])
                    e1t = e1p.tile([128, TPC * NBT * 64], bf16, tag="e1")
                    g0 = 0
                    while g0 < ncols:
                        gw = min(512, ncols - g0)
                        ps1 = psA.tile([128, 512], f32, tag="ps1")
                        nc.tensor.matmul(ps1[:, 0:gw], be1[:], ea_sb[:, g0:g0 + gw],
                                         start=True, stop=True)
                        nc.scalar.activation(e1t[:, g0:g0 + gw], ps1[:, 0:gw],
                                             Act.Relu)
                        g0 += gw

                    # stage 2 (pair matmul -> e natural in PSUM banks of 8 blks)
                    # + msg = hsrc + e, relu on ACT in place
                    msg = msgp.tile([128, NB, DIM], bf16, tag="msg")
                    for v4 in range(nb // 8):
                        pse = psP.tile([128, 8, DIM], f32, tag="pse")
                        for j in range(4):
                            u = 4 * v4 + j
                            nc.tensor.matmul(pse[:, 2 * j:2 * j + 2, :],
                                             e1t[:, 128 * u:128 * (u + 1)],
                                             be2[:], start=True, stop=True)
                        nc.vector.tensor_add(msg[:, 8 * v4:8 * v4 + 8, :],
                                             hsrc[:, 8 * v4:8 * v4 + 8, :], pse[:])
                    nc.scalar.activation(msg[:, 0:nb, :], msg[:, 0:nb, :], Act.Relu)

                    # one-hot generation: t = dstrel - n via PE, is_eq(imm 0)
                    dsl = smallp.tile([8, TPC * NBT // 4, 128], bf16, tag="dsl")
                    g4 = s0 // 512
                    ng = nb // 4
                    nc.sync.dma_start(out=dsl[:, 0:ng, :],
                                      in_=dstrelT_d[:, g4:g4 + ng, :])
                    ohs = []
                    for gq in range(ng):
                        pst4 = psA.tile([128, 512], f32, tag="ps1", name="pst4")
                        nc.tensor.matmul(pst4[:], dsl[:, gq, :], tgen_sb[:],
                                         start=True, stop=True)
                        oh = smallp.tile([128, 512], bf16, tag="oh", bufs=6)
                        nc.vector.tensor_single_scalar(oh[:], pst4[:], 0.0,
                                                       Alu.is_equal)
                        ohs.append(oh)

                    # scatter: per tile, accumulate its blocks into PSUM
                    for k, ti in enumerate(tl):
                        aps = psG.tile([DIM, 128], f32, tag="aggps")
                        blocks = ([k * BA + i for i in range(BA)] +
                                  [ntl * BA + k * BB + i for i in range(BB)])
                        for j, b in enumerate(blocks):
                            nc.tensor.matmul(aps[:], msg[:, b, :],
                                             ohs[b // 4][:, 128 * (b % 4):
                                                         128 * (b % 4) + 128],
                                             start=(j == 0), stop=(j == NBT - 1))
                        nc.vector.tensor_copy(aggT[:, 128 * ti:128 * (ti + 1)],
                                              aps[:])

                # ---- node update
                g0 = 0
                while g0 < NLOC:
                    gw = min(512, NLOC - g0)
                    sl = slice(g0, g0 + gw)
                    zT = smallp.tile([DIM, 512], f32, tag="zT")
                    nc.vector.scalar_tensor_tensor(zT[:, 0:gw], hT[:, sl],
                                                   eps1p[l][:], aggT[:, sl],
                                                   Alu.mult, Alu.add)
                    ps1 = psA.tile([128, 512], f32, tag="ps1")
                    nc.tensor.matmul(ps1[0:DIM, 0:gw], m1[:], zT[:, 0:gw],
                                     start=True, stop=True)
                    r1 = smallp.tile([DIM, 512], f32, tag="r1")
                    nc.scalar.activation(r1[:, 0:gw], ps1[0:DIM, 0:gw], Act.Relu)
                    ps2 = psB.tile([128, 512], f32, tag="ps2")
                    nc.tensor.matmul(ps2[0:DIM, 0:gw], m2[:], r1[:, 0:gw],
                                     start=True, stop=True)
                    rr = smallp.tile([DIM, 512], f32, tag="rr")
                    nc.scalar.activation(rr[:, 0:gw], ps2[0:DIM, 0:gw], Act.Relu)
                    nc.vector.tensor_scalar(hT[:, sl], rr[:, 0:gw], bn_s[l][:],
                                            bn_t[l][:], Alu.mult, Alu.add)
                    g0 += gw
                if _DEBUG and l == 0:
                    nc.sync.dma_start(out=dbg_z[:], in_=hT[:])  # reuse dbg_z for post-MLP hT
                # transpose h^T -> natural, DMA to bounce, AllGather
                bl = bounce[l]
                for ti in range(TILES):
                    pst = psP.tile([128, 2, DIM], f32, tag="pse")
                    nc.tensor.transpose(pst[:, 0, :],
                                        hT[:, 128 * ti:128 * (ti + 1)],
                                        idf_sb[0:DIM, 0:DIM])
                    xn = smallp.tile([128, DIM], f32, tag="xn")
                    nc.vector.tensor_copy(xn[:], pst[:, 0, :])
                    nc.sync.dma_start(out=bl[128 * ti:128 * (ti + 1), :], in_=xn[:])
                    if _DEBUG:
                        nc.sync.dma_start(out=dbg[l][128 * ti:128 * (ti + 1), :],
                                          in_=xn[:])
                if l < NLAYERS - 1:
                    nc.gpsimd.collective_compute(
                        "AllGather", Alu.bypass,
                        replica_groups=[list(range(NCORES))],
                        ins=[bl[:]], outs=[tables[l + 1][:]])

            # ---------------- pooling
            arin_sb = res.tile([128, GWIN, 260], f32, tag="arin")
            nc.vector.memset(arin_sb[:], 0.0)
            pooled_ps = [psA.tile([128, 512], f32, tag="ps1", name="plA"),
                         psB.tile([128, 512], f32, tag="ps2", name="plB"),
                         psP.tile([128, 512], f32, tag="pse", name="plC"),
                         psG.tile([128, 512], f32, tag="aggps", name="plD")]
            for ti in range(TILES):
                cat_t = smallp.tile([128, 260], f32, tag="cat")
                for l in range(NLAYERS):
                    nc.sync.dma_start(out=cat_t[:, 64 * l:64 * (l + 1)],
                                      in_=bounce[l][128 * ti:128 * (ti + 1), :])
                nc.vector.tensor_copy(cat_t[:, 256:257], rm_sb[:, ti:ti + 1])
                nc.vector.memset(cat_t[:, 257:260], 0.0)
                for w in range(GWIN):
                    oh = smallp.tile([128, 128], f32, tag="ohg")
                    nc.vector.tensor_single_scalar(
                        oh[:], iota_sb[:],
                        gid4_sb[:, w * TILES + ti:w * TILES + ti + 1], Alu.is_equal)
                    nc.tensor.matmul(pooled_ps[w][:, 0:257], oh[:], cat_t[:, 0:257],
                                     start=(ti == 0), stop=(ti == TILES - 1))
            for w in range(GWIN):
                nc.vector.tensor_copy(arin_sb[:, w, 0:257], pooled_ps[w][:, 0:257])
            nc.sync.dma_start(out=arin_d[:], in_=arin_sb[:])
            nc.gpsimd.collective_compute(
                "AllReduce", Alu.add, replica_groups=[list(range(NCORES))],
                ins=[arin_d[:]], outs=[arout_d[:]])
            pf = res.tile([128, GWIN, 260], f32, tag="pf")
            nc.sync.dma_start(out=pf[:], in_=arout_d[:])

            # mean + head
            pT = res.tile([128, 2, 512], f32, tag="pT")
            for w in range(GWIN):
                cnt = smallp.tile([128, 1], f32, tag="cnt")
                nc.vector.tensor_scalar_max(cnt[:], pf[:, w, 256:257], 1.0)
                inv = smallp.tile([128, 1], f32, tag="inv")
                nc.vector.reciprocal(inv[:], cnt[:])
                pm = smallp.tile([128, 256], f32, tag="pm")
                nc.vector.tensor_scalar_mul(pm[:], pf[:, w, 0:256], inv[:])
                for k in range(2):
                    pst = psP.tile([128, 512], f32, tag="pse")
                    nc.tensor.transpose(pst[:, 0:128], pm[:, 128 * k:128 * (k + 1)],
                                        idf_sb[:])
                    nc.vector.tensor_copy(pT[:, k, 128 * w:128 * (w + 1)],
                                          pst[:, 0:128])
            hps = psA.tile([128, 512], f32, tag="ps1")
            for k in range(2):
                nc.tensor.matmul(hps[0:DIM, :], fc1w_sb[:, k, :], pT[:, k, :],
                                 start=(k == 0), stop=(k == 1))
            h1 = res.tile([DIM, 512], f32, tag="h1")
            nc.scalar.activation(h1[:], hps[0:DIM, :], Act.Relu,
                                 bias=fcb_sb["b1"][:])
            hps2 = psB.tile([128, 512], f32, tag="ps2")
            nc.tensor.matmul(hps2[0:DIM, :], fc2w_sb[:], h1[:], start=True, stop=True)
            h2 = res.tile([DIM, 512], f32, tag="h2")
            nc.scalar.activation(h2[:], hps2[0:DIM, :], Act.Relu,
                                 bias=fcb_sb["b2"][:])
            hps3 = psA.tile([128, 512], f32, tag="ps1")
            nc.tensor.matmul(hps3[0:DIM, :], fc3w_sb[:], h2[:], start=True, stop=True)
            h3 = res.tile([DIM, 512], f32, tag="h3")
            nc.scalar.activation(h3[:], hps3[0:DIM, :], Act.Relu,
                                 bias=fcb_sb["b3"][:])
            hps4 = psB.tile([128, 512], f32, tag="ps2")
            nc.tensor.matmul(hps4[0:1, :], fc4w_sb[:], h3[:], start=True, stop=True)
            ho = res.tile([1, G], f32, tag="ho")
            nc.scalar.activation(ho[:], hps4[0:1, :], Act.Identity,
                                 bias=fcb_sb["b4"][:])
            nc.sync.dma_start(out=out_d[:], in_=ho[:])

    nc.compile()
    return nc


# ------------------------------------------------------------------ runner --

_CACHE = {}


def kernel(**inputs):
    from concourse.bass_utils import run_bass_kernel_spmd
    in_maps, struct = _prep(inputs)
    key = (struct["BA"], struct["BB"])
    if key not in _CACHE:
        _CACHE[key] = _build(struct)
    nc = _CACHE[key]
    trace = os.environ.get("BASSGIN_TRACE", "0") == "1"
    res = run_bass_kernel_spmd(nc, in_maps, core_ids=list(range(NCORES)),
                               trace=trace)
    kernel.last_result = res
    out = res.results[0]["out"].reshape(G).astype(np.float32)
    return out

